# revision 8
# baseline (speedup 1.0000x reference)
"""Per-edge dot product kernel for Trainium2 (8 NeuronCores).

Computes out[e] = sum(h[src[e]] * h[dst[e]], axis=-1) for
h: [100000, 64] f32, src/dst: [1000000] int indices.

Design (v3 -- SWDGE dma_gather on 4 queues):
  - Edges sharded across 8 cores (125k each). h kept in HBM as
    [4, 25000, 64] f32 pieces (dma_gather idx is int16, so gathers
    address one 25000-row piece).
  - Host buckets each core's edges by (src piece, dst piece) -- 16
    buckets -- and packs per-bucket chunks of up to 896 edges. Both
    sides of a chunk are gathered in the same slot order, so the
    hs/hd tiles align with no on-chip shuffling.
  - Each chunk issues two dma_gather instructions (src piece, dst
    piece), round-robined over the 4 SWDGE queues so Q7 descriptor
    generation (~8.8 ns/row serial, the key bottleneck) runs
    concurrently; measured ~4.1 ns/row at 4 queues. Gathered rows
    land edge-major [128, 7, 64] f32 (row i -> partition i%128,
    slot i//128). Unused slots are padded with idx 0 (discarded).
  - DVE multiplies and reduces over D=64 -> dots [128, 7] f32,
    DMA'd out per chunk. Full f32 precision end to end.
  - Host unsorts chunk slots back to edge order (index bookkeeping
    only; all FLOPs and h movement happen on device).
"""

import sys

import numpy as np

_TRN_REPO = "/opt/trn_rl_repo"
if _TRN_REPO not in sys.path:
    sys.path.insert(0, _TRN_REPO)

N_NODES = 100000
N_EDGES = 1000000
D = 64
N_CORES = 8
E_CORE = N_EDGES // N_CORES   # 125000

NPIECE = 4
WPIECE = 25000                # 4 * 25000 = 100000
NBUCKET = NPIECE * NPIECE     # 16
CH = 896                      # chunk slots (multiple of 128)
M = CH // 128                 # 7 free-dim slots
CH16 = CH // 16               # idx tile free dim

_PROGRAM_CACHE = {}


def _build_program(chunks_per_bucket):
    import concourse.tile as tile
    from concourse import bacc, mybir

    nchunk = NBUCKET * chunks_per_bucket

    nc = bacc.Bacc("TRN2", target_bir_lowering=False, debug=False,
                   num_swdge_queues=4)

    h_t = nc.dram_tensor("h4", [NPIECE, WPIECE, D], mybir.dt.float32,
                         kind="ExternalInput")
    si_t = nc.dram_tensor("src_idx", [nchunk, 128, CH16], mybir.dt.int16,
                          kind="ExternalInput")
    di_t = nc.dram_tensor("dst_idx", [nchunk, 128, CH16], mybir.dt.int16,
                          kind="ExternalInput")
    out_t = nc.dram_tensor("edot", [nchunk, 128, M], mybir.dt.float32,
                           kind="ExternalOutput")

    with tile.TileContext(nc) as tc:
        with (
            tc.tile_pool(name="idx", bufs=32) as ipool,
            tc.tile_pool(name="gat", bufs=32) as gpool,
            tc.tile_pool(name="prd", bufs=10) as ppool,
            tc.tile_pool(name="dot", bufs=12) as dpool,
        ):
            q = 0
            for c in range(nchunk):
                b = c // chunks_per_bucket
                ps, pd = divmod(b, NPIECE)

                si = ipool.tile([128, CH16], mybir.dt.int16, tag="si")
                nc.sync.dma_start(out=si[:], in_=si_t.ap()[c])
                di = ipool.tile([128, CH16], mybir.dt.int16, tag="di")
                nc.scalar.dma_start(out=di[:], in_=di_t.ap()[c])

                hs = gpool.tile([128, M, D], mybir.dt.float32, tag="hs")
                nc.gpsimd.dma_gather(
                    out_ap=hs[:], in_ap=h_t.ap()[ps], idxs_ap=si[:],
                    num_idxs=CH, num_idxs_reg=CH, elem_size=D,
                    queue_num=q % 4,
                )
                q += 1
                hd = gpool.tile([128, M, D], mybir.dt.float32, tag="hd")
                nc.gpsimd.dma_gather(
                    out_ap=hd[:], in_ap=h_t.ap()[pd], idxs_ap=di[:],
                    num_idxs=CH, num_idxs_reg=CH, elem_size=D,
                    queue_num=q % 4,
                )
                q += 1

                prod = ppool.tile([128, M, D], mybir.dt.float32, tag="prod")
                nc.vector.tensor_mul(
                    out=prod[:].rearrange("p m d -> p (m d)"),
                    in0=hs[:].rearrange("p m d -> p (m d)"),
                    in1=hd[:].rearrange("p m d -> p (m d)"),
                )
                dots = dpool.tile([128, M], mybir.dt.float32, tag="dots")
                nc.vector.tensor_reduce(
                    out=dots[:],
                    in_=prod[:],
                    axis=mybir.AxisListType.X,
                    op=mybir.AluOpType.add,
                )
                nc.sync.dma_start(out=out_t.ap()[c], in_=dots[:])

    nc.compile()
    return nc


def _get_program(chunks_per_bucket=9):
    key = chunks_per_bucket
    if key not in _PROGRAM_CACHE:
        _PROGRAM_CACHE[key] = _build_program(chunks_per_bucket)
    return _PROGRAM_CACHE[key]


def _prep_core(src, dst, chunks_per_bucket):
    """Pack one core's edges. Returns (src_idx, dst_idx [nchunk,128,CH16]
    int16, (chunk, part, slot) per edge for output reconstruction)."""
    nchunk = NBUCKET * chunks_per_bucket
    ps = src // WPIECE
    pd = dst // WPIECE
    b = ps * NPIECE + pd
    order = np.argsort(b, kind="stable")
    counts = np.bincount(b, minlength=NBUCKET)

    pos = np.empty(len(src), dtype=np.int64)   # rank within bucket
    off = np.concatenate([[0], np.cumsum(counts)])
    pos[order] = np.arange(len(src)) - off[b[order]]

    chunk = b * chunks_per_bucket + pos // CH
    i = pos % CH                                # slot within chunk
    part = i % 128
    mslot = i // 128

    src_local = (src - ps * WPIECE).astype(np.int16)
    dst_local = (dst - pd * WPIECE).astype(np.int16)

    src_idx = np.zeros((nchunk, 128, CH16), dtype=np.int16)
    dst_idx = np.zeros((nchunk, 128, CH16), dtype=np.int16)
    # wrapped layout: slot i -> [16*core + i%16, i//16], replicated 8 cores
    prow = i % 16
    pfree = i // 16
    for corep in range(8):
        src_idx[chunk, 16 * corep + prow, pfree] = src_local
        dst_idx[chunk, 16 * corep + prow, pfree] = dst_local
    return src_idx, dst_idx, (chunk, part, mslot)


def _run(h, src, dst, trace=False):
    from concourse.bass_utils import run_bass_kernel_spmd

    h = np.ascontiguousarray(np.asarray(h, dtype=np.float32))
    src = np.asarray(src).astype(np.int64)
    dst = np.asarray(dst).astype(np.int64)

    h4 = h.reshape(NPIECE, WPIECE, D)

    # chunk capacity: default 9 per bucket; grow if an input overflows
    cpb = 9
    while True:
        maxcnt = 0
        for c in range(N_CORES):
            sl = slice(c * E_CORE, (c + 1) * E_CORE)
            bb = (src[sl] // WPIECE) * NPIECE + dst[sl] // WPIECE
            maxcnt = max(maxcnt, np.bincount(bb, minlength=NBUCKET).max())
        if maxcnt <= cpb * CH:
            break
        cpb = -(-int(maxcnt) // CH)

    in_maps = []
    metas = []
    for c in range(N_CORES):
        sl = slice(c * E_CORE, (c + 1) * E_CORE)
        src_idx, dst_idx, meta = _prep_core(src[sl], dst[sl], cpb)
        metas.append(meta)
        in_maps.append({"h4": h4, "src_idx": src_idx, "dst_idx": dst_idx})

    nc = _get_program(cpb)
    res = run_bass_kernel_spmd(nc, in_maps, list(range(N_CORES)), trace=trace)

    parts = []
    for c in range(N_CORES):
        dots = np.asarray(res.results[c]["edot"])   # [nchunk, 128, M]
        chunk, part, mslot = metas[c]
        parts.append(dots[chunk, part, mslot])
    return np.concatenate(parts), res


def kernel(h, src, dst):
    out, _ = _run(h, src, dst)
    return out
